# revision 1
# baseline (speedup 1.0000x reference)
"""Trainium2 Bass kernel for LongRangeAttention.

Block-local MHA (8 heads, segment=64) + pooled global MHA (4 heads) over
segment means, broadcast back and summed.

Sharding: 8 cores; core c handles batch b=c//2, token half h=c%2 (2048
tokens = 32 segments). Each core receives its batch element's x feature-major
([D, T]) with the token axis rotated so its own 2048 tokens come first —
global attention over segment means is permutation-equivariant, so each core
computes the pooled attention redundantly in its rotated order and reads off
the outputs for its own (first 32) segments.

All matmuls run as float32r (full fp32 storage, TF32-like PE mode, 1
cycle/row at moving-dim >= 256); the output projection runs bf16 on the
attention output to save SBUF.
"""

import math

import numpy as np

B, T, D = 4, 4096, 1024
SEG = 64
N_CORES = 8
TL = T // 2          # tokens per core
NSEG = T // SEG      # segments per batch element (64)
HL, HDL = 8, 128     # local heads
HG, HDG = 4, 256     # global heads
QUAD = 256           # tokens per inner block
NQ = TL // QUAD      # 8
MASK_VAL = -30000.0
SCL_L = 1.0 / math.sqrt(HDL)
SCL_G = 1.0 / math.sqrt(HDG)

_CACHE = {}


def _split_drain_tile_context():
    """TileContext whose kernel-tail drain spreads its sem waits across SP
    nops — the walrus build here rejects >2 sync waits on CTRL instrs."""
    from bass_rust import N_PROCS
    from concourse import tile as tile_mod
    from concourse.vector_clock import ScopedClock, VectorClock

    class SplitDrainTileContext(tile_mod.TileContext):
        def _drain_and_barrier(self, tick_clock, wait_clock):
            gc = tick_clock.global_clock
            for p in range(N_PROCS):
                if gc[p] > 0:
                    vc = VectorClock(
                        [gc[q] if q == p else 0 for q in range(N_PROCS)]
                    )
                    nop = self.nc.sync.nop(hint=f"drain_split_{p}", nofuse=True)
                    wait_clock.add_sem_waits(nop.ins, ScopedClock({None: vc}))
            # The SP nops above precede the drain in program order, so all
            # sems have reached the global clock before it executes.
            self.nc.sync.drain()
            self.nc.all_engine_barrier()
            popped = self.nc._tile_sem_poison_stack.pop()
            assert popped is self._sem_poison
            self.nc.clear_and_free_semaphores(list(self.sems.allocated().values()))
            self.nc.all_engine_barrier()

    return SplitDrainTileContext


def _fixup_waits(nc, max_waits=2):
    """This walrus build rejects instructions with >2 sync waits. Hoist the
    excess onto same-engine nops inserted just before the instruction —
    program order on the engine preserves the gating semantics."""
    import concourse.mybir as mybir

    ctr = [0]
    for f in nc.m.functions:
        for bb in f.blocks:
            new_insts = []
            for inst in bb.instructions:
                max_waits = 1
                si = inst.sync_info
                waits = list(si.on_wait) if si and si.on_wait else []
                if len(waits) > max_waits:
                    excess, keep = waits[:-max_waits], waits[-max_waits:]
                    for i in range(0, len(excess), max_waits):
                        nop = mybir.InstNoOp(name=f"waitnop{ctr[0]}", ins=[], outs=[])
                        ctr[0] += 1
                        nop.engine = inst.engine
                        nop.sync_info = mybir.SyncInfo(
                            on_wait=excess[i:i + max_waits], on_update=[]
                        )
                        new_insts.append(nop)
                    inst.sync_info = mybir.SyncInfo(
                        on_wait=keep, on_update=si.on_update
                    )
                new_insts.append(inst)
            if len(new_insts) != len(bb.instructions):
                try:
                    bb.instructions = new_insts
                except Exception:
                    bb.instructions[:] = new_insts
    return nc


def _build_nc():
    import concourse.bass as bass
    import concourse.mybir as mybir

    f32 = mybir.dt.float32
    f32r = mybir.dt.float32r
    bf16 = mybir.dt.bfloat16
    X = mybir.AxisListType.X
    Exp = mybir.ActivationFunctionType.Exp
    TC = _split_drain_tile_context()

    nc = bass.Bass()
    dp = nc.declare_dram_parameter
    xT = dp("xT", [D, T], f32r, isOutput=False)
    xTbf = dp("xTbf", [D, T], bf16, isOutput=False)
    wqkT = dp("wqkT", [D, 2 * D], f32r, isOutput=False)
    wvT = dp("wvT", [D, D], f32r, isOutput=False)
    woTbf = dp("woTbf", [D, D], bf16, isOutput=False)
    wgqkTbf = dp("wgqkTbf", [D, 2 * D], bf16, isOutput=False)
    wgvTbf = dp("wgvTbf", [D, D], bf16, isOutput=False)
    wgoTbf = dp("wgoTbf", [D, D], bf16, isOutput=False)
    bqk = dp("bqk", [2 * D, 1], f32, isOutput=False)
    bv = dp("bv", [1, D], f32r, isOutput=False)
    bo = dp("bo", [1, D], f32, isOutput=False)
    bgqk = dp("bgqk", [2 * D, 1], f32, isOutput=False)
    bgvbf = dp("bgvbf", [1, D], bf16, isOutput=False)
    bgo = dp("bgo", [1, D], f32, isOutput=False)
    ident = dp("ident", [128, 128], f32r, isOutput=False)
    identbf = dp("identbf", [SEG, SEG], bf16, isOutput=False)
    mask2 = dp("mask2", [128, 128], f32, isOutput=False)
    bcastbf = dp("bcastbf", [SEG, TL], bf16, isOutput=False)
    ones = dp("ones", [1, 128], f32r, isOutput=False)
    onesbf = dp("onesbf", [1, SEG], bf16, isOutput=False)
    out = dp("out", [TL, D], f32, isOutput=True)

    with TC(nc) as tc:
        with (
            tc.tile_pool(name="const", bufs=1) as cpool,
            tc.tile_pool(name="persist", bufs=1) as ppool,
            tc.tile_pool(name="wl", bufs=1) as wl,
            tc.tile_pool(name="wgs", bufs=4) as wgs,
            tc.tile_pool(name="xmb", bufs=2) as xmb,
            tc.tile_pool(name="lwork", bufs=2) as lw,
            tc.tile_pool(name="ls", bufs=3) as ls,
            tc.tile_pool(name="aop", bufs=2) as aop,
            tc.tile_pool(name="cw", bufs=2) as cw,
            tc.tile_pool(name="ps", bufs=1, space="PSUM") as psp,
        ):
            # ---- local weights + constants first: PE starts ASAP ----
            wqk_sb = [wl.tile([128, 2 * D], f32r, tag=f"wqk{d}", name=f"wqk{d}") for d in range(8)]
            wv_sb = [wl.tile([128, D], f32r, tag=f"wv{d}", name=f"wv{d}") for d in range(8)]
            wo_sb = [wl.tile([128, D], bf16, tag=f"wo{h}", name=f"wo{h}") for h in range(8)]
            for d in range(8):
                nc.sync.dma_start(out=wqk_sb[d][:], in_=wqkT[d * 128:(d + 1) * 128, :])
                nc.sync.dma_start(out=wv_sb[d][:], in_=wvT[d * 128:(d + 1) * 128, :])
                nc.sync.dma_start(out=wo_sb[d][:], in_=woTbf[d * 128:(d + 1) * 128, :])
            ident_sb = cpool.tile([128, 128], f32r, tag="ident", name="ident")
            nc.sync.dma_start(out=ident_sb[:], in_=ident[:])
            identg_sb = cpool.tile([SEG, SEG], bf16, tag="identg", name="identg")
            nc.sync.dma_start(out=identg_sb[:], in_=identbf[:])
            mask_sb = cpool.tile([128, 128], f32, tag="mask", name="mask")
            nc.sync.dma_start(out=mask_sb[:], in_=mask2[:])
            ones_sb = cpool.tile([1, 128], f32r, tag="ones", name="ones")
            nc.sync.dma_start(out=ones_sb[:], in_=ones[:])
            onesbf_sb = cpool.tile([1, SEG], bf16, tag="onesbf", name="onesbf")
            nc.sync.dma_start(out=onesbf_sb[:], in_=onesbf[:])
            bqk_sb = cpool.tile([128, 16], f32, tag="bqk", name="bqk")
            nc.sync.dma_start(
                out=bqk_sb[:], in_=bqk.rearrange("(j p) o -> p (j o)", p=128)
            )
            bv_sb = cpool.tile([1, D], f32r, tag="bv", name="bv")
            nc.sync.dma_start(out=bv_sb[:], in_=bv[:])
            bog_sb = cpool.tile([1, D], f32r, tag="bog", name="bog")
            nc.gpsimd.dma_start(out=bog_sb[:], in_=bo[:])
            nc.gpsimd.dma_start(
                out=bog_sb[:], in_=bgo[:], accum_op=mybir.AluOpType.add
            )
            bgqk_sb = cpool.tile([128, 16], f32, tag="bgqk", name="bgqk")
            nc.sync.dma_start(
                out=bgqk_sb[:], in_=bgqk.rearrange("(j p) o -> p (j o)", p=128)
            )
            bcast_sb = cpool.tile([SEG, TL], bf16, tag="bcast", name="bcast")
            nc.sync.dma_start(out=bcast_sb[:], in_=bcastbf[:])

            # ---- segment means (bf16 side input), feeds global path ----
            sums = [
                ppool.tile([128, NSEG], f32, tag=f"msum{d}", name=f"msum{d}")
                for d in range(8)
            ]
            means = [
                ppool.tile([128, NSEG], bf16, tag=f"means{d}", name=f"means{d}")
                for d in range(8)
            ]
            for d in range(8):
                for blk in range(8):
                    xt = xmb.tile([128, 512], bf16, tag="xm", name="xm")
                    nc.sync.dma_start(
                        out=xt[:],
                        in_=xTbf[d * 128:(d + 1) * 128, blk * 512:(blk + 1) * 512],
                    )
                    nc.vector.reduce_sum(
                        out=sums[d][:, blk * 8:(blk + 1) * 8],
                        in_=xt.rearrange("p (s t) -> p s t", t=SEG),
                        axis=X,
                    )
                nc.vector.tensor_scalar_mul(means[d][:], sums[d][:], 1.0 / SEG)

            qkg = [ppool.tile([128, NSEG], bf16, tag=f"qkg{j}", name=f"qkg{j}") for j in range(16)]
            vg_sb = ppool.tile([SEG, D], bf16, tag="vg", name="vg")
            og = [ppool.tile([128, NSEG], bf16, tag=f"og{j}", name=f"og{j}") for j in range(8)]
            outg_sb = ppool.tile([SEG, D], bf16, tag="outg", name="outg")

            def emit_global():
                # pooled global attention over segment means (all bf16,
                # weights streamed through small tiles)
                for j in range(16):
                    ps = psp.tile([128, NSEG], f32, tag="pss", name="psqkg", bufs=2)
                    for d in range(8):
                        wt = wgs.tile([128, 128], bf16, tag="wgqk", name="wgqk")
                        nc.sync.dma_start(
                            out=wt[:],
                            in_=wgqkTbf[d * 128:(d + 1) * 128, j * 128:(j + 1) * 128],
                        )
                        nc.tensor.matmul(
                            ps[:], lhsT=wt[:], rhs=means[d][:],
                            start=(d == 0), stop=(d == 7),
                        )
                    nc.vector.tensor_scalar_add(qkg[j][:], ps[:], bgqk_sb[:, j:j + 1])
                for nb in range(2):
                    ps = psp.tile([SEG, 512], f32, tag="psv", name="psvg")
                    for d in range(8):
                        wt = wgs.tile([128, 512], bf16, tag="wgv", name="wgv", bufs=2)
                        nc.sync.dma_start(
                            out=wt[:],
                            in_=wgvTbf[d * 128:(d + 1) * 128, nb * 512:(nb + 1) * 512],
                        )
                        nc.tensor.matmul(
                            ps[:], lhsT=means[d][:], rhs=wt[:],
                            start=(d == 0), stop=False,
                        )
                    bgv_t = wgs.tile([1, 512], bf16, tag="bgvt", name="bgvt")
                    nc.sync.dma_start(
                        out=bgv_t[:], in_=bgvbf[0:1, nb * 512:(nb + 1) * 512]
                    )
                    nc.tensor.matmul(
                        ps[:], lhsT=onesbf_sb[0:1, 0:SEG], rhs=bgv_t[:],
                        start=False, stop=True,
                    )
                    nc.vector.tensor_copy(vg_sb[:, nb * 512:(nb + 1) * 512], ps[:])
                for hg in range(HG):
                    ps_s = psp.tile([SEG, SEG], f32, tag="pss", name="psgs", bufs=2)
                    for c in range(2):
                        j = hg * 2 + c
                        nc.tensor.matmul(
                            ps_s[:], lhsT=qkg[j][:], rhs=qkg[8 + j][:],
                            start=(c == 0), stop=(c == 1),
                        )
                    s_sb = cw.tile([SEG, SEG], f32, tag="gs", name="gs")
                    nc.vector.tensor_copy(s_sb[:], ps_s[:])
                    nm = cw.tile([SEG, 1], f32, tag="gnm", name="gnm")
                    nc.vector.reduce_max(out=nm[:], in_=s_sb[:], axis=X, negate=True)
                    nc.vector.tensor_scalar_mul(nm[:], nm[:], SCL_G)
                    Pg = cw.tile([SEG, SEG], bf16, tag="gP", name="gP")
                    gsum = cw.tile([SEG, 1], f32, tag="gsum", name="gsum")
                    nc.scalar.activation(
                        Pg[:], s_sb[:], Exp, bias=nm[:], scale=SCL_G,
                        accum_out=gsum[:],
                    )
                    gr = cw.tile([SEG, 1], f32, tag="gr", name="gr")
                    nc.vector.reciprocal(gr[:], gsum[:])
                    nc.vector.tensor_scalar_mul(Pg[:], Pg[:], gr[:])
                    ps_t = psp.tile([SEG, SEG], bf16, tag="pst", name="psgt")
                    nc.tensor.transpose(ps_t[:], Pg[:], identg_sb[:])
                    PTg = cw.tile([SEG, SEG], bf16, tag="gPT", name="gPT")
                    nc.vector.tensor_copy(PTg[:], ps_t[:])
                    for c in range(2):
                        j = hg * 2 + c
                        ps_o = psp.tile([128, SEG], f32, tag="psa", name="psgo")
                        nc.tensor.matmul(
                            ps_o[:],
                            lhsT=vg_sb[:, j * 128:(j + 1) * 128],
                            rhs=PTg[:],
                            start=True, stop=True,
                        )
                        nc.vector.tensor_copy(og[j][:], ps_o[:])
                for nb in range(2):
                    ps = psp.tile([SEG, 512], f32, tag="psv", name="psog")
                    for h in range(8):
                        wt = wgs.tile([128, 512], bf16, tag="wgo", name="wgo", bufs=2)
                        nc.sync.dma_start(
                            out=wt[:],
                            in_=wgoTbf[h * 128:(h + 1) * 128, nb * 512:(nb + 1) * 512],
                        )
                        nc.tensor.matmul(
                            ps[:], lhsT=og[h][:], rhs=wt[:],
                            start=(h == 0), stop=(h == 7),
                        )
                    nc.vector.tensor_copy(outg_sb[:, nb * 512:(nb + 1) * 512], ps[:])

            def emit_quad_attention(q):
                """qk/v projection + block-local attention for one quad.
                Returns the pair attention-output tiles (bf16)."""
                xq = []
                for d in range(8):
                    t_ = lw.tile([128, QUAD], f32r, tag=f"xq{d}", name=f"xq{d}")
                    nc.sync.dma_start(
                        out=t_[:],
                        in_=xT[d * 128:(d + 1) * 128, q * QUAD:(q + 1) * QUAD],
                    )
                    xq.append(t_)
                qk = []
                for j in range(16):
                    ps = psp.tile([128, QUAD], f32, tag="psqk", name="psqk", bufs=2)
                    for d in range(8):
                        nc.tensor.matmul(
                            ps[:],
                            lhsT=wqk_sb[d][:, j * 128:(j + 1) * 128],
                            rhs=xq[d][:],
                            start=(d == 0),
                            stop=(d == 7),
                        )
                    t_ = lw.tile([128, QUAD], f32r, tag=f"qk{j}", name=f"qk{j}", bufs=1)
                    nc.vector.tensor_scalar_add(t_[:], ps[:], bqk_sb[:, j:j + 1])
                    qk.append(t_)
                v = []
                for tt in range(2):
                    t_ = lw.tile([128, D], f32r, tag=f"v{tt}", name=f"v{tt}", bufs=1)
                    for nb in range(2):
                        ps = psp.tile([128, 512], f32, tag="psv", name="psv")
                        for d in range(8):
                            nc.tensor.matmul(
                                ps[:],
                                lhsT=xq[d][:, tt * 128:(tt + 1) * 128],
                                rhs=wv_sb[d][:, nb * 512:(nb + 1) * 512],
                                start=(d == 0),
                                stop=False,
                            )
                        nc.tensor.matmul(
                            ps[:],
                            lhsT=ones_sb[0:1, 0:128],
                            rhs=bv_sb[0:1, nb * 512:(nb + 1) * 512],
                            start=False,
                            stop=True,
                        )
                        nc.vector.tensor_copy(t_[:, nb * 512:(nb + 1) * 512], ps[:])
                    v.append(t_)
                aos = []
                for pp in range(2):
                    col0 = pp * 128
                    ao = []
                    for h in range(HL):
                        ps_s = psp.tile([128, 128], f32, tag="pss", name="pss", bufs=2)
                        nc.tensor.matmul(
                            ps_s[:],
                            lhsT=qk[h][:, col0:col0 + 128],
                            rhs=qk[8 + h][:, col0:col0 + 128],
                            start=True,
                            stop=True,
                        )
                        s_sb = ls.tile([128, 128], f32, tag="s", name="s")
                        nc.vector.tensor_add(s_sb[:], ps_s[:], mask_sb[:])
                        nm = ls.tile([128, 1], f32, tag="nm", name="nm")
                        nc.vector.reduce_max(
                            out=nm[:], in_=s_sb[:], axis=X, negate=True
                        )
                        nc.vector.tensor_scalar_mul(nm[:], nm[:], SCL_L)
                        P = ls.tile([128, 128], f32r, tag="P", name="P")
                        ssum = ls.tile([128, 1], f32, tag="ssum", name="ssum")
                        nc.scalar.activation(
                            P[:], s_sb[:], Exp, bias=nm[:], scale=SCL_L,
                            accum_out=ssum[:],
                        )
                        rr = ls.tile([128, 1], f32, tag="rr", name="rr")
                        nc.vector.reciprocal(rr[:], ssum[:])
                        nc.vector.tensor_scalar_mul(P[:], P[:], rr[:])
                        ps_t = psp.tile([128, 128], f32r, tag="pst", name="pst")
                        nc.tensor.transpose(ps_t[:], P[:], ident_sb[:])
                        PT = ls.tile([128, 128], f32r, tag="PT", name="PT")
                        nc.vector.tensor_copy(PT[:], ps_t[:])
                        ps_a = psp.tile([128, 128], f32, tag="psa", name="psa")
                        nc.tensor.matmul(
                            ps_a[:],
                            lhsT=v[pp][:, h * 128:(h + 1) * 128],
                            rhs=PT[:],
                            start=True,
                            stop=True,
                        )
                        t_ = aop.tile(
                            [128, 128], bf16, tag=f"ao{h}", name=f"ao{h}", bufs=5
                        )
                        nc.vector.tensor_copy(t_[:], ps_a[:])
                        ao.append(t_)
                    aos.append(ao)
                return aos

            def emit_outproj(q, pp, ao):
                tok0 = q * QUAD + pp * 128
                for nb in range(2):
                    ps_o = psp.tile([128, 512], f32, tag="pso", name="pso")
                    for h in range(8):
                        nc.tensor.matmul(
                            ps_o[:],
                            lhsT=ao[h][:],
                            rhs=wo_sb[h][:, nb * 512:(nb + 1) * 512],
                            start=(h == 0),
                            stop=False,
                        )
                    nc.tensor.matmul(
                        ps_o[:],
                        lhsT=bcast_sb[:, tok0:tok0 + 128],
                        rhs=outg_sb[:, nb * 512:(nb + 1) * 512],
                        start=False,
                        stop=False,
                    )
                    nc.tensor.matmul(
                        ps_o[:],
                        lhsT=ones_sb[0:1, 0:128],
                        rhs=bog_sb[0:1, nb * 512:(nb + 1) * 512],
                        start=False,
                        stop=True,
                    )
                    osb = ls.tile([128, 512], f32, tag="osb", name="osb", bufs=2)
                    nc.vector.tensor_copy(osb[:], ps_o[:])
                    nc.sync.dma_start(
                        out=out[tok0:tok0 + 128, nb * 512:(nb + 1) * 512],
                        in_=osb[:],
                    )

            # quads 0-1 run before the global phase in the PE stream (their
            # output projection is deferred until outg exists); the global
            # phase then slots in while quads 2+ attention continues.
            pending = []
            for q in range(2):
                aos = emit_quad_attention(q)
                pending.append((q, aos))
            emit_global()
            for q, aos in pending:
                for pp in range(2):
                    emit_outproj(q, pp, aos[pp])
            for q in range(2, NQ):
                aos = emit_quad_attention(q)
                for pp in range(2):
                    emit_outproj(q, pp, aos[pp])
    return _fixup_waits(nc)


def _shard_inputs(inputs):
    """Build the 8 per-core input maps from the full problem inputs."""
    import ml_dtypes

    f = np.float32
    bf = ml_dtypes.bfloat16
    x = np.asarray(inputs["x"], f)
    w_in_l = np.asarray(inputs["w_in_local"], f)
    b_in_l = np.asarray(inputs["b_in_local"], f)
    w_out_l = np.asarray(inputs["w_out_local"], f)
    b_out_l = np.asarray(inputs["b_out_local"], f)
    w_in_g = np.asarray(inputs["w_in_global"], f)
    b_in_g = np.asarray(inputs["b_in_global"], f)
    w_out_g = np.asarray(inputs["w_out_global"], f)
    b_out_g = np.asarray(inputs["b_out_global"], f)

    common = {
        "wqkT": np.ascontiguousarray(w_in_l[: 2 * D].T),
        "wvT": np.ascontiguousarray(w_in_l[2 * D:].T),
        "woTbf": np.ascontiguousarray(w_out_l.T).astype(bf),
        "wgqkTbf": np.ascontiguousarray(w_in_g[: 2 * D].T).astype(bf),
        "wgvTbf": np.ascontiguousarray(w_in_g[2 * D:].T).astype(bf),
        "wgoTbf": np.ascontiguousarray(w_out_g.T).astype(bf),
        "bqk": np.ascontiguousarray(b_in_l[: 2 * D].reshape(2 * D, 1)),
        "bv": np.ascontiguousarray(b_in_l[2 * D:].reshape(1, D)),
        "bo": np.ascontiguousarray(b_out_l.reshape(1, D)),
        "bgqk": np.ascontiguousarray(b_in_g[: 2 * D].reshape(2 * D, 1)),
        "bgvbf": np.ascontiguousarray(b_in_g[2 * D:].reshape(1, D)).astype(bf),
        "bgo": np.ascontiguousarray(b_out_g.reshape(1, D)),
        "ident": np.eye(128, dtype=f),
        "identbf": np.eye(SEG, dtype=f).astype(bf),
        "mask2": _mask2(),
        "bcastbf": _bcast().astype(bf),
        "ones": np.ones((1, 128), f),
        "onesbf": np.ones((1, SEG), f).astype(bf),
    }
    in_maps = []
    for c in range(N_CORES):
        b, h = divmod(c, 2)
        xT_b = np.ascontiguousarray(x[b].T)  # [D, T]
        if h == 1:
            xT_b = np.ascontiguousarray(
                np.concatenate([xT_b[:, TL:], xT_b[:, :TL]], axis=1)
            )
        in_maps.append({"xT": xT_b, "xTbf": xT_b.astype(bf), **common})
    return in_maps


def _mask2():
    m = np.full((128, 128), MASK_VAL, np.float32)
    m[:64, :64] = 0.0
    m[64:, 64:] = 0.0
    return m


def _bcast():
    m = np.zeros((SEG, TL), np.float32)
    for t in range(TL):
        m[t // SEG, t] = 1.0
    return m


def _get_runtime():
    """Compile once; return (jitted sharded fn, names metadata)."""
    if "rt" in _CACHE:
        return _CACHE["rt"]
    import jax
    import concourse.mybir as mybir
    from concourse import bass2jax
    from jax.experimental.shard_map import shard_map
    from jax.sharding import Mesh, PartitionSpec

    nc = _build_nc()
    bass2jax.install_neuronx_cc_hook()

    partition_name = nc.partition_id_tensor.name if nc.partition_id_tensor else None
    in_names, out_names, out_avals = [], [], []
    for alloc in nc.m.functions[0].allocations:
        if not isinstance(alloc, mybir.MemoryLocationSet):
            continue
        name = alloc.memorylocations[0].name
        if alloc.kind == "ExternalInput":
            if name != partition_name:
                in_names.append(name)
        elif alloc.kind == "ExternalOutput":
            shape = tuple(alloc.tensor_shape)
            dtype = mybir.dt.np(alloc.dtype)
            out_names.append(name)
            out_avals.append(jax.core.ShapedArray(shape, dtype))
    n_params = len(in_names)
    all_in_names = in_names + out_names
    if partition_name is not None:
        all_in_names = all_in_names + [partition_name]

    def _body(*args):
        operands = list(args)
        if partition_name is not None:
            operands.append(bass2jax.partition_id_tensor())
        outs = bass2jax._bass_exec_p.bind(
            *operands,
            out_avals=tuple(out_avals),
            in_names=tuple(all_in_names),
            out_names=tuple(out_names),
            lowering_input_output_aliases=(),
            sim_require_finite=True,
            sim_require_nnan=True,
            nc=nc,
        )
        return tuple(outs)

    devices = jax.devices()[:N_CORES]
    mesh = Mesh(np.asarray(devices), ("core",))
    in_specs = (PartitionSpec("core"),) * (n_params + len(out_names))
    out_specs = (PartitionSpec("core"),) * len(out_names)
    sharded = jax.jit(
        shard_map(
            _body, mesh=mesh, in_specs=in_specs, out_specs=out_specs, check_rep=False
        ),
        keep_unused=True,
    )
    rt = {
        "nc": nc,
        "sharded": sharded,
        "in_names": in_names,
        "out_names": out_names,
        "out_avals": out_avals,
        "dbg_name": nc.dbg_addr.name if nc.dbg_addr is not None else None,
    }
    _CACHE["rt"] = rt
    return rt


def _concat_args(rt, in_maps):
    """Stack per-core inputs along axis 0 (global view for shard_map)."""
    args = []
    for name in rt["in_names"]:
        if name == rt["dbg_name"]:
            args.append(np.zeros((N_CORES, 2), np.uint32))
            continue
        args.append(np.concatenate([np.asarray(m[name]) for m in in_maps], axis=0))
    for av in rt["out_avals"]:
        args.append(np.zeros((N_CORES * av.shape[0], *av.shape[1:]), av.dtype))
    return args


def _run(in_maps):
    rt = _get_runtime()
    if rt["dbg_name"] is not None:
        for m in in_maps:
            m.setdefault(rt["dbg_name"], np.zeros((1, 2), np.uint32))
    args = _concat_args(rt, in_maps)
    outs = rt["sharded"](*args)
    return [np.asarray(o) for o in outs]


def kernel(**inputs):
    in_maps = _shard_inputs(inputs)
    outs = _run(in_maps)
    out_global = outs[0]  # [8*TL, D]; core c rows [c*TL, (c+1)*TL)
    return out_global.reshape(B, T, D).astype(np.float32)

